# revision 12
# baseline (speedup 1.0000x reference)
"""Diagonal-matrix multiply kernel for Trainium2: y = x * |diagonal_|.

Full input x is (65536, 1024) f32; diagonal_ is (1024,) f32.
Data-parallel across 8 NeuronCores: each core processes 8192 contiguous
rows of x; the diagonal is replicated to every core (sharding is just a
contiguous row split, so the global x never needs rearranging on host).

Per-core kernel (Tile framework):
  - broadcast-DMA diagonal_ into a [128, 1024] SBUF tile (partition
    stride 0), take |d| once with tensor_scalar(abs_max, 0).
  - loop over 16 tiles of [128 partitions x 4096 free] (4 consecutive
    rows per partition -> each DMA moves 2 MiB contiguous HBM).
  - vector-engine tensor_mul against the broadcast |d| tile.
  - store back with the mirrored access pattern.

Execution goes through the bass_exec PJRT primitive (the axon-tunneled
path) with the jitted callable cached, so repeated kernel() calls do not
recompile.
"""

import numpy as np
import jax
from jax.sharding import Mesh, NamedSharding, PartitionSpec
from jax.experimental.shard_map import shard_map

import concourse.bass as bass
import concourse.tile as tile
from concourse import mybir
from concourse.bass2jax import (
    _bass_exec_p,
    install_neuronx_cc_hook,
    partition_id_tensor,
)

N_CORES = 8
ROWS, COLS = 65536, 1024
SHARD = ROWS // N_CORES  # 8192 rows per core
P = 128                  # SBUF partitions
R = 4                    # consecutive rows packed into one partition line
FREE = R * COLS          # 4096 f32 = 16 KiB per partition line
NTILES = SHARD // (P * R)  # 16
BUFS = 8                 # in-flight tile buffers (128 KiB/partition)


def _build_nc(reps: int = 1) -> bass.Bass:
    nc = bass.Bass()
    x = nc.dram_tensor("x", [SHARD, COLS], mybir.dt.float32, kind="ExternalInput")
    d = nc.dram_tensor("diagonal_", [COLS], mybir.dt.float32, kind="ExternalInput")
    y = nc.dram_tensor("y", [SHARD, COLS], mybir.dt.float32, kind="ExternalOutput")

    # row index = (n*P + p)*R + r: tile n, partition p holds R consecutive
    # rows (16 KiB contiguous per partition line).
    xv = x[:].rearrange("(n p r) m -> n p (r m)", p=P, r=R)
    yv = y[:].rearrange("(n p r) m -> n p (r m)", p=P, r=R)

    d_ap = d[:]
    d_bcast = bass.AP(
        tensor=d_ap.tensor,
        offset=d_ap.offset,
        ap=[[0, P], d_ap.ap[0]],
    )
    total = reps * NTILES

    # Raw bass (no Tile): this walrus build only allows one sync-wait per
    # compute instruction, so all waits are standalone sequencer ops and
    # every dma/compute carries at most a single then_inc update.
    with (
        nc.sbuf_tensor([P, COLS], mybir.dt.float32) as draw,
        nc.sbuf_tensor([P, COLS], mybir.dt.float32) as negd,
        nc.sbuf_tensor([P, COLS], mybir.dt.float32) as absd,
        nc.sbuf_tensor([P, BUFS, FREE], mybir.dt.float32) as xt,
        nc.semaphore("ld_sem") as ld,
        nc.semaphore("vs_sem") as vs,
        nc.semaphore("st_sem") as st,
        nc.Block() as block,
    ):
        absd3 = absd[:, None, :].broadcast_to((P, R, COLS))

        @block.sync
        def _(sync):
            # loads on the SP engine's HWDGE ring
            sync.dma_start(out=draw[:], in_=d_bcast).then_inc(ld, 16)
            for t in range(total):
                n = t % NTILES
                if t >= BUFS:
                    # slot reuse: the store that drained this slot is done
                    sync.wait_ge(st, 16 * (t - BUFS + 1))
                sync.dma_start(out=xt[:, t % BUFS, :], in_=xv[n]).then_inc(ld, 16)

        @block.vector
        def _(vector):
            vector.wait_ge(ld, 16)
            # |d| = max(d, -d); own-sem waits drain the DVE pipeline
            # between dependent ops (no same-engine interlock on DVE)
            vector.tensor_scalar_mul(
                out=negd[:], in0=draw[:], scalar1=-1.0
            ).then_inc(vs, 1)
            vector.wait_ge(vs, 1)
            vector.tensor_max(out=absd[:], in0=draw[:], in1=negd[:]).then_inc(vs, 1)
            vector.wait_ge(vs, 2)
            for t in range(total):
                vector.wait_ge(ld, 16 * (t + 2))
                x3 = xt[:, t % BUFS, :].rearrange("p (r m) -> p r m", r=R)
                vector.tensor_mul(x3, x3, absd3).then_inc(vs, 1)

        @block.scalar
        def _(scalar):
            # stores on the ACT engine's HWDGE ring (separate from loads)
            for t in range(total):
                n = t % NTILES
                scalar.wait_ge(vs, t + 3)
                scalar.dma_start(out=yv[n], in_=xt[:, t % BUFS, :]).then_inc(st, 16)

    return nc


class _Runner:
    """Caches the Bass module + jitted shard_map callable for one config."""

    def __init__(self, reps: int = 1):
        install_neuronx_cc_hook()
        self.nc = _build_nc(reps)
        nc = self.nc
        assert nc.dbg_addr is None

        in_names = ["x", "diagonal_"]
        out_names = ["y"]
        out_avals = [jax.core.ShapedArray((SHARD, COLS), np.float32)]
        all_names = in_names + out_names
        partition_name = (
            nc.partition_id_tensor.name if nc.partition_id_tensor else None
        )
        if partition_name is not None:
            all_names = all_names + [partition_name]

        def _body(*args):
            operands = list(args)
            if partition_name is not None:
                operands.append(partition_id_tensor())
            return tuple(
                _bass_exec_p.bind(
                    *operands,
                    out_avals=tuple(out_avals),
                    in_names=tuple(all_names),
                    out_names=tuple(out_names),
                    lowering_input_output_aliases=(),
                    sim_require_finite=True,
                    sim_require_nnan=True,
                    nc=nc,
                )
            )

        devices = jax.devices()[:N_CORES]
        assert len(devices) == N_CORES, f"need {N_CORES} cores, have {len(devices)}"
        self.mesh = Mesh(np.asarray(devices), ("core",))
        spec = PartitionSpec("core")
        self.sharding = NamedSharding(self.mesh, spec)
        n_args = len(in_names) + len(out_names)
        self.fn = jax.jit(
            shard_map(
                _body,
                mesh=self.mesh,
                in_specs=(spec,) * n_args,
                out_specs=(spec,) * len(out_names),
                check_rep=False,
            ),
            donate_argnums=(2,),  # the zero-filled output buffer
            keep_unused=True,
        )

    def globals_from_inputs(self, x: np.ndarray, diagonal_: np.ndarray):
        x = np.ascontiguousarray(x, dtype=np.float32)
        diagonal_ = np.ascontiguousarray(diagonal_, dtype=np.float32)
        d_global = np.tile(diagonal_, N_CORES)  # (8192,), one copy per core
        zeros = np.zeros((ROWS, COLS), dtype=np.float32)
        return x, d_global, zeros

    def __call__(self, x_global, d_global, zeros):
        return self.fn(x_global, d_global, zeros)[0]


_RUNNERS: dict[int, _Runner] = {}


def _get_runner(reps: int = 1) -> _Runner:
    if reps not in _RUNNERS:
        _RUNNERS[reps] = _Runner(reps)
    return _RUNNERS[reps]


def kernel(x: np.ndarray, diagonal_: np.ndarray) -> np.ndarray:
    r = _get_runner(1)
    y = r(*r.globals_from_inputs(x, diagonal_))
    return np.asarray(y)
